# revision 3
# baseline (speedup 1.0000x reference)
"""Causal self-attention (flipped mask: attend to k >= q) on 8 Trainium2 cores.

Sharding: 2-way data parallel over batch x 4-way head parallel (4 heads/core).
Each core computes x[b] -> qkv (its 4 heads) -> attention -> partial out-proj
(its 256 rows of Wo); the host sums the 4 partials per batch (tensor-parallel
unshard) to produce the full [B, T, C] output; the out-proj bias is added on
the host during the unshard sum.

v3 structure (per core):
  - x transposed on the HOST; xT [C, T] f16 DMA'd straight to SBUF in 4
    t-slabs matching phase-B consumption order.
  - phase B reordered: all kT chains first, then qT(g0,m0), so that the
    (g=0,n=0) attention scores + softmax exp PREFILL under the tail of
    phase B (ACT is the phase-C bottleneck; B is PE-bound).
  - scores for the even/odd head of a pair land in ONE [128,1024] 2-bank
    PSUM tile; exp is ONE scalar-engine instruction per k-tile.
  - no additive mask: scores are O(1) so exp never overflows f16; the
    diagonal 128-block gets a 0/1 triangular f16 multiply (DVE 2x) and the
    fully-masked strip a gpsimd memset; band exp width is trimmed.
  - softmax denominator folded into the AV matmul via a ones column in v;
    reciprocal via DMA-reshape; normalization on DVE.
  - phase D (out-proj) interleaved per q-chunk n: PSUM from the score pool,
    PSUM->SBUF f16 copy on DVE, per-t-tile f16 DMA out (hides out traffic).
"""

import numpy as np

B, T, C = 2, 2048, 1024
H = 16
D = 64
NH = 4           # heads per core
HC = NH * D      # 256 local head cols
SCALE = 0.125    # 1/sqrt(D)
N_CORES = 8

NT = T // 128    # 16 t-tiles
NCC = C // 128   # 8 c-chunks
NQ = T // 512    # 4 q-chunks of 512
NJ = T // 128    # 16 kt-chunks of 128

_CACHE = {}


def _build_nc():
    import concourse.tile as tile
    from concourse import bacc, mybir

    f32 = mybir.dt.float32
    f16 = mybir.dt.float16
    Exp = mybir.ActivationFunctionType.Exp
    Ident = mybir.ActivationFunctionType.Identity

    nc = bacc.Bacc(None, target_bir_lowering=False, debug=False)

    xbT = nc.dram_tensor("xbT", [C, T], f16, kind="ExternalInput")
    wq = nc.dram_tensor("wq", [C, HC], f16, kind="ExternalInput")
    wk = nc.dram_tensor("wk", [C, HC], f16, kind="ExternalInput")
    wv = nc.dram_tensor("wv", [C, HC], f16, kind="ExternalInput")
    bqs = nc.dram_tensor("bqs", [HC], f32, kind="ExternalInput")
    bk = nc.dram_tensor("bk", [HC], f32, kind="ExternalInput")
    bvb = nc.dram_tensor("bvb", [128, HC], f32, kind="ExternalInput")
    wo = nc.dram_tensor("wo", [HC, C], f16, kind="ExternalInput")
    tri01 = nc.dram_tensor("tri01", [128, 128], f16, kind="ExternalInput")
    out = nc.dram_tensor("out", [T, C], f16, kind="ExternalOutput")

    with tile.TileContext(nc) as tc, (
        tc.tile_pool(name="consts", bufs=1)) as consts, (
        tc.tile_pool(name="wts", bufs=1)) as wts, (
        tc.tile_pool(name="persist", bufs=1)) as persist:

        # ---- weights needed at phase-B start ----
        wq_sb = wts.tile([128, NCC, HC], f16)
        nc.sync.dma_start(out=wq_sb, in_=wq.rearrange("(a p) n -> p a n", p=128))
        wk_sb = wts.tile([128, NCC, HC], f16)
        nc.sync.dma_start(out=wk_sb, in_=wk.rearrange("(a p) n -> p a n", p=128))
        wv_sb = wts.tile([128, NCC, HC], f16)
        nc.sync.dma_start(out=wv_sb, in_=wv.rearrange("(a p) n -> p a n", p=128))

        # ---- x in 4 t-slabs, phase-B consumption order ----
        xT_sb = persist.tile([128, NCC, T], f16)
        xTr = xbT.rearrange("(a p) t -> p a t", p=128)
        for m in range(NQ):
            nc.sync.dma_start(
                out=xT_sb[:, :, m * 512:(m + 1) * 512],
                in_=xTr[:, :, m * 512:(m + 1) * 512],
            )

        # ---- small consts; wo last (only needed by phase D) ----
        tri_sb = consts.tile([128, 128], f16)
        nc.sync.dma_start(out=tri_sb, in_=tri01[:, :])
        bq_sb = consts.tile([128, 2], f32)
        nc.sync.dma_start(out=bq_sb, in_=bqs.rearrange("(a p) -> p a", p=128))
        bk_sb = consts.tile([128, 2], f32)
        nc.sync.dma_start(out=bk_sb, in_=bk.rearrange("(a p) -> p a", p=128))
        bvb_sb = consts.tile([128, NH, D], f32)
        nc.sync.dma_start(out=bvb_sb, in_=bvb.rearrange("p (h d) -> p h d", h=NH))
        wo_sb = wts.tile([128, 2, C], f16)
        nc.sync.dma_start(out=wo_sb, in_=wo.rearrange("(a p) n -> p a n", p=128))

        # ---- persistent activations ----
        qT_sb = persist.tile([128, 2, T], f16)   # [2 head-pair chunks, T]
        kT_sb = persist.tile([128, 2, T], f16)
        # v, augmented: per t-tile, per pair g: [65 even | 130 odd]
        # even block: cols 0..63 = v(2g), col 64 = 1.0
        # odd block:  col 0 = 1.0 (tile col 65), cols 64..127 = v(2g+1)
        v_sb = persist.tile([128, NT, 2, 195], f16)
        yT_sb = persist.tile([128, 2, T], f16)

        # ones columns for the folded softmax denominator (cols 66..128 and
        # 193..194 feed junk output partitions that are never read)
        for t0 in range(NT):
            nc.gpsimd.memset(v_sb[:, t0, :, 64:66], 1.0)

        with (
            tc.tile_pool(name="epool", bufs=18) as epool,
            tc.tile_pool(name="rpool", bufs=2) as rpool,
            tc.tile_pool(name="opool", bufs=2) as opool,
            tc.tile_pool(name="psS", bufs=2, space="PSUM") as psS,
        ):
            # -------- phase C score+exp emitter (one k-tile j, pair g) -----
            def emit_score_exp(g, n, j):
                qs = n * 512
                b_i = j - 4 * n
                ks = j * 128
                ps = psS.tile([128, 1024], f32, tag="s", name="ps")
                nc.tensor.matmul(
                    ps[:, 0:512],
                    lhsT=(kT_sb[0:64, g, ks:ks + 128]),
                    rhs=(qT_sb[0:64, g, qs:qs + 512]),
                    start=True, stop=True,
                )
                nc.tensor.matmul(
                    ps[:, 512:1024],
                    lhsT=(kT_sb[64:128, g, ks:ks + 128]),
                    rhs=(qT_sb[64:128, g, qs:qs + 512]),
                    start=True, stop=True,
                )
                e = epool.tile([128, 1024], f16, tag="e", name="e")
                if b_i < 4:
                    w = 128 * (b_i + 1)
                    e3 = e.rearrange("p (h q) -> p h q", h=2)
                    ps3 = ps.rearrange("p (h q) -> p h q", h=2)
                    nc.scalar.activation(e3[:, :, 0:w], ps3[:, :, 0:w], Exp)
                    nc.vector.tensor_mul(e[:, w - 128:w], e[:, w - 128:w], tri_sb)
                    nc.vector.tensor_mul(
                        e[:, 512 + w - 128:512 + w], e[:, 512 + w - 128:512 + w],
                        tri_sb,
                    )
                    if w < 512:
                        nc.gpsimd.memset(e3[:, :, w:512], 0.0)
                else:
                    nc.scalar.activation(e, ps, Exp)
                return e

            def emit_av(g, n, j, e):
                yt = yt_cur[0]
                nc.tensor.matmul(
                    yt[0:65, 0:512],
                    lhsT=(v_sb[:, j, g, 0:65]),
                    rhs=(e[:, 0:512]),
                    start=(j == 4 * n), stop=(j == NJ - 1),
                )
                nc.tensor.matmul(
                    yt[:, 512:1024],
                    lhsT=(v_sb[:, j, g, 65:193]),
                    rhs=(e[:, 512:1024]),
                    start=(j == 4 * n), stop=(j == NJ - 1),
                )

            def emit_normalize(g, n):
                # denominators sit on 1 PSUM partition each (even @64 of the
                # low bank, odd @0 of the high bank).  DMA-reshape [1,512] ->
                # [128,4] to make the reciprocal cheap, then DMA +
                # partition-broadcast 1/sum back across partitions.
                yt = yt_cur[0]
                ye = yt[:, 0:512]
                yo = yt[:, 512:1024]
                qs = n * 512
                tmp = rpool.tile([128, 512], f32, tag="tmp", name="tmp")
                nc.vector.tensor_copy(tmp[64:65, :], ye[64:65, :])
                nc.vector.tensor_copy(tmp[0:1, :], yo[0:1, :])
                rs = rpool.tile([128, 8], f32, tag="rs", name="rs")
                nc.sync.dma_start(out=rs[:, 0:4], in_=tmp[64:65, :])
                nc.sync.dma_start(out=rs[:, 4:8], in_=tmp[0:1, :])
                rr = rpool.tile([128, 8], f32, tag="rr", name="rr")
                nc.vector.reciprocal(rr, rs)
                rt = rpool.tile([128, 1024], f32, tag="rt", name="rt")
                nc.sync.dma_start(out=rt[0:1, 0:512], in_=rr[:, 0:4])
                nc.sync.dma_start(out=rt[0:1, 512:1024], in_=rr[:, 4:8])
                bsbE = rpool.tile([128, 512], f32, tag="bsbE", name="bsbE")
                bsbO = rpool.tile([128, 512], f32, tag="bsbO", name="bsbO")
                nc.gpsimd.partition_broadcast(bsbE[:, :], rt[0:1, 0:512])
                nc.gpsimd.partition_broadcast(bsbO[:, :], rt[0:1, 512:1024])
                nc.vector.tensor_mul(
                    yT_sb[0:64, g, qs:qs + 512], ye[0:64, :], bsbE[0:64, :]
                )
                nc.vector.tensor_mul(
                    yT_sb[64:128, g, qs:qs + 512], yo[64:128, :], bsbO[64:128, :]
                )

            # -------- phase B: projections (kT chains first) --------
            # chain order: k(g0,m0..3), k(g1,m0..3), q(g0,m0), then the rest;
            # after chain 8 the (g0,n0) scores+exps prefill into the stream.
            chain_specs = (
                [(g, m, 1) for g in range(2) for m in range(NQ)]
                + [(0, 0, 0)]
                + [(0, m, 0) for m in range(1, NQ)]
                + [(1, m, 0) for m in range(NQ)]
            )
            prefill = []  # (j, e) for (g=0, n=0)

            with tc.tile_pool(name="psB", bufs=4, space="PSUM") as psB:
                for i, (g, m, is_k) in enumerate(chain_specs):
                    w_sb = wk_sb if is_k else wq_sb
                    t0 = i
                    psqk = psB.tile([128, 512], f32, tag="pj", name="psqk")
                    psv = psB.tile([128, 512], f32, tag="pj", name="psv")
                    for c0 in range(NCC):
                        nc.tensor.matmul(
                            psqk,
                            lhsT=(w_sb[:, c0, g * 128:(g + 1) * 128]),
                            rhs=(xT_sb[:, c0, m * 512:(m + 1) * 512]),
                            start=(c0 == 0), stop=(c0 == NCC - 1),
                        )
                        nc.tensor.matmul(
                            psv[:, 0:HC],
                            lhsT=(xT_sb[:, c0, t0 * 128:(t0 + 1) * 128]),
                            rhs=(wv_sb[:, c0, :]),
                            start=(c0 == 0), stop=(c0 == NCC - 1),
                        )
                    if is_k:
                        nc.scalar.activation(
                            kT_sb[:, g, m * 512:(m + 1) * 512], psqk, Ident,
                            bias=bk_sb[:, g:g + 1], scale=1.0,
                        )
                    else:
                        nc.scalar.activation(
                            qT_sb[:, g, m * 512:(m + 1) * 512], psqk, Ident,
                            bias=bq_sb[:, g:g + 1], scale=SCALE,
                        )
                    psv4 = psv[:, 0:HC].rearrange("p (h d) -> p h d", h=NH)
                    for gg in range(2):
                        nc.vector.tensor_add(
                            v_sb[:, t0, gg, 0:64], psv4[:, 2 * gg, :],
                            bvb_sb[:, 2 * gg, :],
                        )
                        nc.vector.tensor_add(
                            v_sb[:, t0, gg, 129:193], psv4[:, 2 * gg + 1, :],
                            bvb_sb[:, 2 * gg + 1, :],
                        )
                    # prefill (g0,n0) scores into the B tail
                    if i >= 9:
                        take = 2 if i < 15 else 16 - len(prefill)
                        for _ in range(take):
                            j = len(prefill)
                            prefill.append((j, emit_score_exp(0, 0, j)))

            # -------- phases C/D interleaved --------
            with tc.tile_pool(name="psY", bufs=2, space="PSUM") as psY:
                yt_cur = [None]

                def run_group(g, n, pre=None):
                    yt_cur[0] = psY.tile([128, 1024], f32, tag="y", name="yt")
                    if pre is not None:
                        for j, e in pre:
                            emit_av(g, n, j, e)
                    else:
                        lag = []
                        for j in range(4 * n, NJ):
                            e = emit_score_exp(g, n, j)
                            if len(lag) >= 2:
                                emit_av(g, n, *lag.pop(0))
                            lag.append((j, e))
                        for item in lag:
                            emit_av(g, n, *item)
                    emit_normalize(g, n)

                for n in range(NQ):
                    run_group(0, n, pre=prefill if n == 0 else None)
                    run_group(1, n)
                    # phase D for this q-chunk: both pairs' yT ready
                    for t0 in range(4 * n, 4 * n + 4):
                        o_sb = opool.tile([128, C], f16, tag="o", name="o_sb")
                        pd = psS.tile([128, 1024], f32, tag="s", name="pd")
                        for g in range(2):
                            nc.tensor.matmul(
                                pd[:, 0:512],
                                lhsT=(yT_sb[:, g, t0 * 128:(t0 + 1) * 128]),
                                rhs=(wo_sb[:, g, 0:512]),
                                start=(g == 0), stop=(g == 1),
                            )
                            nc.tensor.matmul(
                                pd[:, 512:1024],
                                lhsT=(yT_sb[:, g, t0 * 128:(t0 + 1) * 128]),
                                rhs=(wo_sb[:, g, 512:1024]),
                                start=(g == 0), stop=(g == 1),
                            )
                        nc.vector.tensor_copy(o_sb, pd)
                        nc.sync.dma_start(
                            out=out[t0 * 128:(t0 + 1) * 128, :], in_=o_sb
                        )

    nc.compile()
    return nc


def _host_consts():
    p = np.arange(128)[:, None]
    c = np.arange(128)[None, :]
    tri01 = (p >= c).astype(np.float16)
    return tri01


def make_in_maps(x, Wqkv, bqkv, Wo, bo):
    x = np.asarray(x, dtype=np.float32)
    Wqkv = np.asarray(Wqkv, dtype=np.float32)
    bqkv = np.asarray(bqkv, dtype=np.float32)
    Wo = np.asarray(Wo, dtype=np.float32)
    tri01 = _host_consts()
    xT = [np.ascontiguousarray(x[b].T).astype(np.float16) for b in range(B)]
    in_maps = []
    for core in range(N_CORES):
        b, hg = divmod(core, 4)
        s = HC * hg
        in_maps.append({
            "xbT": xT[b],
            "wq": np.ascontiguousarray(Wqkv[:, s:s + HC]).astype(np.float16),
            "wk": np.ascontiguousarray(Wqkv[:, C + s:C + s + HC]).astype(np.float16),
            "wv": np.ascontiguousarray(Wqkv[:, 2 * C + s:2 * C + s + HC]).astype(np.float16),
            "bqs": np.ascontiguousarray(bqkv[s:s + HC]) * np.float32(SCALE),
            "bk": np.ascontiguousarray(bqkv[C + s:C + s + HC]),
            "bvb": np.ascontiguousarray(
                np.broadcast_to(bqkv[2 * C + s:2 * C + s + HC], (128, HC))
            ),
            "wo": np.ascontiguousarray(Wo[s:s + HC, :]).astype(np.float16),
            "tri01": tri01,
        })
    return in_maps


def unshard(results, bo=None):
    out = np.empty((B, T, C), dtype=np.float32)
    for b in range(B):
        acc = results[4 * b]["out"].astype(np.float32)
        for hg in range(1, 4):
            acc = acc + results[4 * b + hg]["out"].astype(np.float32)
        if bo is not None:
            acc = acc + np.asarray(bo, dtype=np.float32)
        out[b] = acc
    return out


def get_nc():
    if "nc" not in _CACHE:
        _CACHE["nc"] = _build_nc()
    return _CACHE["nc"]


def kernel(x, Wqkv, bqkv, Wo, bo):
    from concourse.bass_utils import run_bass_kernel_spmd

    nc = get_nc()
    in_maps = make_in_maps(x, Wqkv, bqkv, Wo, bo)
    res = run_bass_kernel_spmd(nc, in_maps, list(range(N_CORES)))
    return unshard(res.results, bo=bo)


# revision 5
# speedup vs baseline: 1.0843x; 1.0843x over previous
"""Causal self-attention (flipped mask: attend to k >= q) on 8 Trainium2 cores.

Sharding: 2-way data parallel over batch x 4-way head parallel (4 heads/core).
Each core computes x[b] -> qkv (its 4 heads) -> attention -> partial out-proj
(its 256 rows of Wo); the host sums the 4 partials per batch (tensor-parallel
unshard) to produce the full [B, T, C] output; the out-proj bias is added on
the host during the unshard sum.

v3 structure (per core):
  - x transposed on the HOST; xT [C, T] f16 DMA'd straight to SBUF in 4
    t-slabs matching phase-B consumption order.
  - phase B reordered: all kT chains first, then qT(g0,m0), so that the
    (g=0,n=0) attention scores + softmax exp PREFILL under the tail of
    phase B (ACT is the phase-C bottleneck; B is PE-bound).
  - scores for the even/odd head of a pair land in ONE [128,1024] 2-bank
    PSUM tile; exp is ONE scalar-engine instruction per k-tile.
  - no additive mask: scores are O(1) so exp never overflows f16; the
    diagonal 128-block gets a 0/1 triangular f16 multiply (DVE 2x) and the
    fully-masked strip a gpsimd memset; band exp width is trimmed.
  - softmax denominator folded into the AV matmul via a ones column in v;
    reciprocal via DMA-reshape; normalization on DVE.
  - phase D (out-proj) interleaved per q-chunk n: PSUM from the score pool,
    PSUM->SBUF f16 copy on DVE, per-t-tile f16 DMA out (hides out traffic).
"""

import numpy as np

B, T, C = 2, 2048, 1024
H = 16
D = 64
NH = 4           # heads per core
HC = NH * D      # 256 local head cols
SCALE = 0.125    # 1/sqrt(D)
N_CORES = 8

NT = T // 128    # 16 t-tiles
NCC = C // 128   # 8 c-chunks
NQ = T // 512    # 4 q-chunks of 512
NJ = T // 128    # 16 kt-chunks of 128

_CACHE = {}


def _build_nc():
    import concourse.tile as tile
    from concourse import bacc, mybir

    f32 = mybir.dt.float32
    f16 = mybir.dt.float16
    Exp = mybir.ActivationFunctionType.Exp
    Ident = mybir.ActivationFunctionType.Identity

    nc = bacc.Bacc(None, target_bir_lowering=False, debug=False)

    xbT = nc.dram_tensor("xbT", [C, T], f16, kind="ExternalInput")
    wq = nc.dram_tensor("wq", [C, HC], f16, kind="ExternalInput")
    wk = nc.dram_tensor("wk", [C, HC], f16, kind="ExternalInput")
    wv = nc.dram_tensor("wv", [C, HC], f16, kind="ExternalInput")
    bqs = nc.dram_tensor("bqs", [HC], f32, kind="ExternalInput")
    bk = nc.dram_tensor("bk", [HC], f32, kind="ExternalInput")
    bvb = nc.dram_tensor("bvb", [128, HC], f32, kind="ExternalInput")
    wo = nc.dram_tensor("wo", [HC, C], f16, kind="ExternalInput")
    tri01 = nc.dram_tensor("tri01", [128, 128], f16, kind="ExternalInput")
    out = nc.dram_tensor("out", [T, C], f16, kind="ExternalOutput")

    with tile.TileContext(nc) as tc, (
        tc.tile_pool(name="consts", bufs=1)) as consts, (
        tc.tile_pool(name="wts", bufs=1)) as wts, (
        tc.tile_pool(name="persist", bufs=1)) as persist:

        # ---- weights needed at phase-B start ----
        wq_sb = wts.tile([128, NCC, HC], f16)
        nc.sync.dma_start(out=wq_sb, in_=wq.rearrange("(a p) n -> p a n", p=128))
        wk_sb = wts.tile([128, NCC, HC], f16)
        nc.sync.dma_start(out=wk_sb, in_=wk.rearrange("(a p) n -> p a n", p=128))
        wv_sb = wts.tile([128, NCC, HC], f16)
        nc.sync.dma_start(out=wv_sb, in_=wv.rearrange("(a p) n -> p a n", p=128))

        # ---- x in 4 t-slabs, phase-B consumption order ----
        xT_sb = persist.tile([128, NCC, T], f16)
        xTr = xbT.rearrange("(a p) t -> p a t", p=128)
        # slab 0 split per c-chunk so the first projection chain can start
        # as soon as ~128KB has landed
        for c0 in range(NCC):
            nc.sync.dma_start(
                out=xT_sb[:, c0, 0:512], in_=xTr[:, c0, 0:512]
            )
        for m in range(1, NQ):
            nc.sync.dma_start(
                out=xT_sb[:, :, m * 512:(m + 1) * 512],
                in_=xTr[:, :, m * 512:(m + 1) * 512],
            )

        # ---- small consts; wo last (only needed by phase D) ----
        tri_sb = consts.tile([128, 128], f16)
        nc.sync.dma_start(out=tri_sb, in_=tri01[:, :])
        bq_sb = consts.tile([128, 2], f32)
        nc.sync.dma_start(out=bq_sb, in_=bqs.rearrange("(a p) -> p a", p=128))
        bk_sb = consts.tile([128, 2], f32)
        nc.sync.dma_start(out=bk_sb, in_=bk.rearrange("(a p) -> p a", p=128))
        bvb_sb = consts.tile([128, NH, D], f32)
        nc.sync.dma_start(out=bvb_sb, in_=bvb.rearrange("p (h d) -> p h d", h=NH))
        wo_sb = wts.tile([128, 2, C], f16)
        nc.sync.dma_start(out=wo_sb, in_=wo.rearrange("(a p) n -> p a n", p=128))

        # ---- persistent activations ----
        qT_sb = persist.tile([128, 2, T], f16)   # [2 head-pair chunks, T]
        kT_sb = persist.tile([128, 2, T], f16)
        # v, augmented: per t-tile, per pair g: [65 even | 130 odd]
        # even block: cols 0..63 = v(2g), col 64 = 1.0
        # odd block:  col 0 = 1.0 (tile col 65), cols 64..127 = v(2g+1)
        v_sb = persist.tile([128, NT, 2, 195], f16)
        yT_sb = persist.tile([128, 2, T], f16)

        # ones columns for the folded softmax denominator (cols 66..128 and
        # 193..194 feed junk output partitions that are never read)
        for t0 in range(NT):
            nc.gpsimd.memset(v_sb[:, t0, :, 64:66], 1.0)

        with (
            tc.tile_pool(name="epool", bufs=18) as epool,
            tc.tile_pool(name="rpool", bufs=2) as rpool,
            tc.tile_pool(name="opool", bufs=2) as opool,
            tc.tile_pool(name="psS", bufs=2, space="PSUM") as psS,
        ):
            # -------- phase C score+exp emitter (one k-tile j, pair g) -----
            def emit_score_exp(g, n, j):
                qs = n * 512
                b_i = j - 4 * n
                ks = j * 128
                ps = psS.tile([128, 1024], f32, tag="s", name="ps")
                nc.tensor.matmul(
                    ps[:, 0:512],
                    lhsT=(kT_sb[0:64, g, ks:ks + 128]),
                    rhs=(qT_sb[0:64, g, qs:qs + 512]),
                    start=True, stop=True,
                )
                nc.tensor.matmul(
                    ps[:, 512:1024],
                    lhsT=(kT_sb[64:128, g, ks:ks + 128]),
                    rhs=(qT_sb[64:128, g, qs:qs + 512]),
                    start=True, stop=True,
                )
                e = epool.tile([128, 1024], f16, tag="e", name="e")
                if b_i < 4:
                    w = 128 * (b_i + 1)
                    e3 = e.rearrange("p (h q) -> p h q", h=2)
                    ps3 = ps.rearrange("p (h q) -> p h q", h=2)
                    nc.scalar.activation(e3[:, :, 0:w], ps3[:, :, 0:w], Exp)
                    nc.vector.tensor_mul(e[:, w - 128:w], e[:, w - 128:w], tri_sb)
                    nc.vector.tensor_mul(
                        e[:, 512 + w - 128:512 + w], e[:, 512 + w - 128:512 + w],
                        tri_sb,
                    )
                    if w < 512:
                        nc.gpsimd.memset(e3[:, :, w:512], 0.0)
                else:
                    nc.scalar.activation(e, ps, Exp)
                return e

            def emit_av(g, n, j, e):
                yt = yt_cur[0]
                nc.tensor.matmul(
                    yt[0:65, 0:512],
                    lhsT=(v_sb[:, j, g, 0:65]),
                    rhs=(e[:, 0:512]),
                    start=(j == 4 * n), stop=(j == NJ - 1),
                )
                nc.tensor.matmul(
                    yt[:, 512:1024],
                    lhsT=(v_sb[:, j, g, 65:193]),
                    rhs=(e[:, 512:1024]),
                    start=(j == 4 * n), stop=(j == NJ - 1),
                )

            def emit_normalize(g, n):
                # denominators sit on 1 PSUM partition each (even @64 of the
                # low bank, odd @0 of the high bank).  DMA-reshape [1,512] ->
                # [128,4] to make the reciprocal cheap, then DMA +
                # partition-broadcast 1/sum back across partitions.
                yt = yt_cur[0]
                ye = yt[:, 0:512]
                yo = yt[:, 512:1024]
                qs = n * 512
                tmp = rpool.tile([128, 512], f32, tag="tmp", name="tmp")
                nc.vector.tensor_copy(tmp[64:65, :], ye[64:65, :])
                nc.vector.tensor_copy(tmp[0:1, :], yo[0:1, :])
                rs = rpool.tile([128, 8], f32, tag="rs", name="rs")
                nc.sync.dma_start(out=rs[:, 0:4], in_=tmp[64:65, :])
                nc.sync.dma_start(out=rs[:, 4:8], in_=tmp[0:1, :])
                rr = rpool.tile([128, 8], f32, tag="rr", name="rr")
                nc.vector.reciprocal(rr, rs)
                rt = rpool.tile([128, 1024], f32, tag="rt", name="rt")
                nc.sync.dma_start(out=rt[0:1, 0:512], in_=rr[:, 0:4])
                nc.sync.dma_start(out=rt[0:1, 512:1024], in_=rr[:, 4:8])
                bsbE = rpool.tile([128, 512], f32, tag="bsbE", name="bsbE")
                bsbO = rpool.tile([128, 512], f32, tag="bsbO", name="bsbO")
                nc.gpsimd.partition_broadcast(bsbE[:, :], rt[0:1, 0:512])
                nc.gpsimd.partition_broadcast(bsbO[:, :], rt[0:1, 512:1024])
                nc.vector.tensor_mul(
                    yT_sb[0:64, g, qs:qs + 512], ye[0:64, :], bsbE[0:64, :]
                )
                nc.vector.tensor_mul(
                    yT_sb[64:128, g, qs:qs + 512], yo[64:128, :], bsbO[64:128, :]
                )

            # -------- phase B: projections (kT chains first) --------
            # chain order: k(g0,m0..3), k(g1,m0..3), q(g0,m0), then the rest;
            # after chain 8 the (g0,n0) scores+exps prefill into the stream.
            chain_specs = (
                [(g, m, 1) for g in range(2) for m in range(NQ)]
                + [(0, 0, 0)]
                + [(0, m, 0) for m in range(1, NQ)]
                + [(1, m, 0) for m in range(NQ)]
            )
            prefill = []  # (j, e) for (g=0, n=0)

            with tc.tile_pool(name="psB", bufs=4, space="PSUM") as psB:
                for i, (g, m, is_k) in enumerate(chain_specs):
                    w_sb = wk_sb if is_k else wq_sb
                    t0 = i
                    psqk = psB.tile([128, 512], f32, tag="pj", name="psqk")
                    psv = psB.tile([128, 512], f32, tag="pj", name="psv")
                    for c0 in range(NCC):
                        nc.tensor.matmul(
                            psqk,
                            lhsT=(w_sb[:, c0, g * 128:(g + 1) * 128]),
                            rhs=(xT_sb[:, c0, m * 512:(m + 1) * 512]),
                            start=(c0 == 0), stop=(c0 == NCC - 1),
                        )
                        nc.tensor.matmul(
                            psv[:, 0:HC],
                            lhsT=(xT_sb[:, c0, t0 * 128:(t0 + 1) * 128]),
                            rhs=(wv_sb[:, c0, :]),
                            start=(c0 == 0), stop=(c0 == NCC - 1),
                        )
                    if is_k:
                        nc.scalar.activation(
                            kT_sb[:, g, m * 512:(m + 1) * 512], psqk, Ident,
                            bias=bk_sb[:, g:g + 1], scale=1.0,
                        )
                    else:
                        nc.scalar.activation(
                            qT_sb[:, g, m * 512:(m + 1) * 512], psqk, Ident,
                            bias=bq_sb[:, g:g + 1], scale=SCALE,
                        )
                    psv4 = psv[:, 0:HC].rearrange("p (h d) -> p h d", h=NH)
                    for gg in range(2):
                        nc.vector.tensor_add(
                            v_sb[:, t0, gg, 0:64], psv4[:, 2 * gg, :],
                            bvb_sb[:, 2 * gg, :],
                        )
                        nc.vector.tensor_add(
                            v_sb[:, t0, gg, 129:193], psv4[:, 2 * gg + 1, :],
                            bvb_sb[:, 2 * gg + 1, :],
                        )
                    # prefill (g0,n0) scores into the B tail
                    if i >= 9:
                        take = 2 if i < 15 else 16 - len(prefill)
                        for _ in range(take):
                            j = len(prefill)
                            prefill.append((j, emit_score_exp(0, 0, j)))

            # -------- phases C/D interleaved --------
            with tc.tile_pool(name="psY", bufs=2, space="PSUM") as psY:
                yt_cur = [None]

                def run_group(g, n, pre=None):
                    yt_cur[0] = psY.tile([128, 1024], f32, tag="y", name="yt")
                    if pre is not None:
                        for j, e in pre:
                            emit_av(g, n, j, e)
                    else:
                        lag = []
                        for j in range(4 * n, NJ):
                            e = emit_score_exp(g, n, j)
                            if len(lag) >= 2:
                                emit_av(g, n, *lag.pop(0))
                            lag.append((j, e))
                        for item in lag:
                            emit_av(g, n, *item)
                    emit_normalize(g, n)

                def emit_d(n):
                    # out-projection for q-chunk n; evacuate PSUM on DVE for
                    # early n (ACT exp-heavy) and on ACT for late n (ACT idle)
                    for t0 in range(4 * n, 4 * n + 4):
                        o_sb = opool.tile([128, C], f16, tag="o", name="o_sb")
                        pd = psS.tile([128, 1024], f32, tag="s", name="pd")
                        for g in range(2):
                            nc.tensor.matmul(
                                pd[:, 0:512],
                                lhsT=(yT_sb[:, g, t0 * 128:(t0 + 1) * 128]),
                                rhs=(wo_sb[:, g, 0:512]),
                                start=(g == 0), stop=(g == 1),
                            )
                            nc.tensor.matmul(
                                pd[:, 512:1024],
                                lhsT=(yT_sb[:, g, t0 * 128:(t0 + 1) * 128]),
                                rhs=(wo_sb[:, g, 512:1024]),
                                start=(g == 0), stop=(g == 1),
                            )
                        if n < 2:
                            nc.vector.tensor_copy(o_sb, pd)
                        else:
                            nc.scalar.activation(o_sb, pd, Ident)
                        nc.sync.dma_start(
                            out=out[t0 * 128:(t0 + 1) * 128, :], in_=o_sb
                        )

                # D(n) is emitted one group after (1,n) so its matmuls never
                # wait on the normalize latency chain of group (1,n)
                for n in range(NQ):
                    run_group(0, n, pre=prefill if n == 0 else None)
                    if n >= 1:
                        emit_d(n - 1)
                    run_group(1, n)
                emit_d(NQ - 1)

    nc.compile()
    return nc


def _host_consts():
    p = np.arange(128)[:, None]
    c = np.arange(128)[None, :]
    tri01 = (p >= c).astype(np.float16)
    return tri01


def make_in_maps(x, Wqkv, bqkv, Wo, bo):
    x = np.asarray(x, dtype=np.float32)
    Wqkv = np.asarray(Wqkv, dtype=np.float32)
    bqkv = np.asarray(bqkv, dtype=np.float32)
    Wo = np.asarray(Wo, dtype=np.float32)
    tri01 = _host_consts()
    xT = [np.ascontiguousarray(x[b].T).astype(np.float16) for b in range(B)]
    in_maps = []
    for core in range(N_CORES):
        b, hg = divmod(core, 4)
        s = HC * hg
        in_maps.append({
            "xbT": xT[b],
            "wq": np.ascontiguousarray(Wqkv[:, s:s + HC]).astype(np.float16),
            "wk": np.ascontiguousarray(Wqkv[:, C + s:C + s + HC]).astype(np.float16),
            "wv": np.ascontiguousarray(Wqkv[:, 2 * C + s:2 * C + s + HC]).astype(np.float16),
            "bqs": np.ascontiguousarray(bqkv[s:s + HC]) * np.float32(SCALE),
            "bk": np.ascontiguousarray(bqkv[C + s:C + s + HC]),
            "bvb": np.ascontiguousarray(
                np.broadcast_to(bqkv[2 * C + s:2 * C + s + HC], (128, HC))
            ),
            "wo": np.ascontiguousarray(Wo[s:s + HC, :]).astype(np.float16),
            "tri01": tri01,
        })
    return in_maps


def unshard(results, bo=None):
    out = np.empty((B, T, C), dtype=np.float32)
    for b in range(B):
        acc = results[4 * b]["out"].astype(np.float32)
        for hg in range(1, 4):
            acc = acc + results[4 * b + hg]["out"].astype(np.float32)
        if bo is not None:
            acc = acc + np.asarray(bo, dtype=np.float32)
        out[b] = acc
    return out


def get_nc():
    if "nc" not in _CACHE:
        _CACHE["nc"] = _build_nc()
    return _CACHE["nc"]


def kernel(x, Wqkv, bqkv, Wo, bo):
    from concourse.bass_utils import run_bass_kernel_spmd

    nc = get_nc()
    in_maps = make_in_maps(x, Wqkv, bqkv, Wo, bo)
    res = run_bass_kernel_spmd(nc, in_maps, list(range(N_CORES)))
    return unshard(res.results, bo=bo)
